# revision 32
# baseline (speedup 1.0000x reference)
"""GNN edge-softmax attention kernel for 8 Trainium2 NeuronCores.

Strategy (4 src-rows x 2 dst-halves core grid):
  - Host routes each edge to core (row(src), half(dst)). Nodes are packed
    into 128-node tiles balanced by edge count; each tile's edges are padded
    to whole 128-edge blocks so every core runs an identical program.
  - Host precomputes the edge-bias term eb = edges @ Wb.T + bb (tiny linear
    map) so the device never touches the raw 64-dim edge features.
  - Per core: project q/k/v slices with PE into f16 tables (khv table
    assembled across the quad with a 2-chunk AllGather overlapped with the
    qh projection), then for each gather batch: pipelined dma_gather
    (prepare_only + trigger_dma, so descriptor prep never blocks on DMA),
    per-tile one-hot matmuls for qh rows, batched DVE mult/reduce for the
    per-edge logits, exp on ACT, and scatter-accumulate num/den into PSUM
    with selection-matrix matmuls.
  - num/den partials are written in f16 and pair-ReduceScattered in two
    chunks (first chunk overlaps the main loop); each core normalizes and
    applies the output projection for its half of the tiles. Host adds bo.
"""

import math
import sys

import numpy as np

sys.path.insert(0, "/opt/trn_rl_repo")

import concourse.bacc as bacc
import concourse.bass as bass
import concourse.mybir as mybir
import concourse.tile as tile
from concourse import bass_utils

F16 = mybir.dt.float16
F8 = mybir.dt.float8e4
F32 = mybir.dt.float32
I16 = mybir.dt.int16

H = 8            # heads
D = 16           # head dim
TD = H * D       # 128
QD = 256         # q/k/v feature dim
PD = 64          # edge pair feature dim
R = 4            # src rows of the core grid
C = 2            # dst cols of the core grid
P = 128

AF = mybir.ActivationFunctionType
ALU = mybir.AluOpType


def _wrap16(idx: np.ndarray) -> np.ndarray:
    """dma_gather index layout: [128, n/16] with idx i at (i%16 + 16k, i//16)."""
    n = idx.shape[0]
    assert n % 16 == 0
    w = idx.reshape(n // 16, 16).T.astype(np.int16)  # [16, n/16]
    return np.tile(w, (8, 1))  # replicate across the 8 partition groups


def prepare(q, k, v, edges, edge_index, Wq, Wk, Wv, Wb, bb, Wo, bo):
    N = q.shape[0]
    E = edges.shape[0]
    ntiles_row = math.ceil(N / (R * P))          # tiles per src row
    NROW = ntiles_row * P                        # nodes per row (padded)
    NPAD = NROW * R
    DHALF = NPAD // 2                            # dst-half size
    assert DHALF < 32768, "dst half must fit int16"

    src = np.asarray(edge_index[:, 0], dtype=np.int64)
    dst = np.asarray(edge_index[:, 1], dtype=np.int64)
    deg = np.bincount(src, minlength=N)

    # --- greedy node->tile packing balanced by edge count ---
    T = R * ntiles_row
    order = np.argsort(-deg, kind="stable")
    tile_cnt = np.zeros(T, dtype=np.int64)       # nodes in tile
    tile_edges = np.zeros(T, dtype=np.int64)
    node_tile = np.zeros(N, dtype=np.int32)
    node_slot = np.zeros(N, dtype=np.int32)
    import heapq
    heap = [(0, t) for t in range(T)]
    heapq.heapify(heap)
    for n in order:
        while True:
            e_cnt, t = heapq.heappop(heap)
            if tile_cnt[t] < P:
                break
        node_tile[n] = t
        node_slot[n] = tile_cnt[t]
        tile_cnt[t] += 1
        tile_edges[t] += deg[n]
        if tile_cnt[t] < P:
            heapq.heappush(heap, (tile_edges[t], t))

    row_of_edge = node_tile[src] // ntiles_row
    j_of_edge = (dst // DHALF).astype(np.int64)
    tloc_of_edge = (node_tile[src] % ntiles_row).astype(np.int64)

    # per (core, tile_local) edge counts -> shared block counts per tile slot
    core_of_edge = row_of_edge * C + j_of_edge
    cnt = np.zeros((R * C, ntiles_row), dtype=np.int64)
    np.add.at(cnt, (core_of_edge, tloc_of_edge), 1)
    bpt = np.maximum(1, np.ceil(cnt.max(axis=0) / P).astype(np.int64))  # [ntiles_row]
    blk_off = np.concatenate([[0], np.cumsum(bpt)])   # block offset per tile
    NBLK = int(blk_off[-1])
    ECAP = NBLK * P

    # host-side edge bias: eb[e, h] = edges @ Wb.T + bb
    eb_all = (np.asarray(edges, np.float32) @ np.asarray(Wb, np.float32).T
              + np.asarray(bb, np.float32)[None, :]).astype(np.float16)  # [E, H]

    # --- per-core edge arrays ---
    cores = []
    for core in range(R * C):
        i, j = core // C, core % C
        mask = core_of_edge == core
        es, ed, et = src[mask], dst[mask], tloc_of_edge[mask]
        # order edges by tile slot
        ordr = np.argsort(et, kind="stable")
        es, ed, et = es[ordr], ed[ordr], et[ordr]
        # positions: per tile, fill from blk_off[t]*P
        pos = np.zeros(len(es), dtype=np.int64)
        start = 0
        for t in range(ntiles_row):
            c = int((et == t).sum())
            pos[start:start + c] = blk_off[t] * P + np.arange(c)
            start += c
        eidx = np.nonzero(mask)[0][ordr]

        import ml_dtypes
        F8NP = ml_dtypes.float8_e4m3
        dst_local = np.zeros(ECAP, dtype=np.int16)
        src_rel = np.full(ECAP, 255, dtype=np.int64)
        ebE = np.zeros((ECAP, H), dtype=np.float16)
        dst_local[pos] = (ed - j * DHALF).astype(np.int16)
        src_rel[pos] = node_slot[es]
        ebE[pos] = eb_all[eidx]
        # eb in edge-major block layout [128, NBLK*H]
        ebT = np.ascontiguousarray(
            ebE.reshape(NBLK, P, H).transpose(1, 0, 2)).reshape(P, NBLK * H)
        # one-hot selection matrices (fp8, exact 0/1)
        S_en = np.zeros((ECAP, P), dtype=F8NP)
        valid = src_rel < P
        S_en[np.nonzero(valid)[0], src_rel[valid]] = 1.0
        S_en3 = S_en.reshape(NBLK, P, P)                       # [b, e, n]
        S_mat = np.ascontiguousarray(S_en3.transpose(1, 0, 2)).reshape(P, ECAP)   # [e_part, (b n)]
        ST_mat = np.ascontiguousarray(S_en3.transpose(2, 0, 1)).reshape(P, ECAP)  # [n_part, (b e)]

        # constants: this core projects ALL of half j's khv table locally
        # (no AllGather: collectives on the gpsimd queue block descriptor
        # generation, which is the kernel's scarcest resource)
        qlo = j * DHALF
        qhi = min(qlo + DHALF, N)
        kT = np.zeros((QD, DHALF), dtype=np.float16)
        vT = np.zeros((QD, DHALF), dtype=np.float16)
        if qhi > qlo:
            kT[:, :qhi - qlo] = np.asarray(k[qlo:qhi], np.float32).T.astype(np.float16)
            vT[:, :qhi - qlo] = np.asarray(v[qlo:qhi], np.float32).T.astype(np.float16)
        # q rows permuted into (tile_local, slot) order for this row i
        qT = np.zeros((QD, NROW), dtype=np.float16)
        rmask = node_tile // ntiles_row == i
        rn = np.nonzero(rmask)[0]
        qpos = (node_tile[rn] % ntiles_row) * P + node_slot[rn]
        qT[:, qpos] = np.asarray(q[rn], np.float32).T.astype(np.float16)

        cores.append(dict(
            dst_idx=_wrap16(dst_local), S_mat=S_mat, ST_mat=ST_mat,
            ebT=ebT, kT=kT, vT=vT, qT=qT,
        ))

    norm = D ** -0.5
    consts = dict(
        WkT=np.asarray(Wk, np.float32).T.astype(np.float16),
        WvT=np.asarray(Wv, np.float32).T.astype(np.float16),
        WqT=(np.asarray(Wq, np.float32) * norm).T.astype(np.float16),
        WoT=np.asarray(Wo, np.float32).T.astype(np.float16),
        identity=np.eye(P, dtype=np.float16),
    )
    meta = dict(N=N, NPAD=NPAD, NROW=NROW, DHALF=DHALF, ntiles_row=ntiles_row,
                NBLK=NBLK, ECAP=ECAP, bpt=bpt.tolist(), blk_off=blk_off.tolist(),
                node_tile=node_tile, node_slot=node_slot, deg=deg)
    return cores, consts, meta


def build_program(meta, gather_batch=3):
    """Build the SPMD bass program. Returns compiled nc."""
    ntr = meta["ntiles_row"]
    NROW, DHALF = meta["NROW"], meta["DHALF"]
    NBLK, ECAP = meta["NBLK"], meta["ECAP"]
    bpt, blk_off = meta["bpt"], meta["blk_off"]
    NQ = NROW // P        # qh chunks
    # ReduceScatter split: chunk A covers tiles [0, H1), chunk B the rest.
    H1 = ((ntr // 2 + 1) // 2) * 2  # even tile count near the middle
    HA, HB = H1 // 2, (ntr - H1) // 2
    assert H1 % 2 == 0 and (ntr - H1) % 2 == 0

    GB = gather_batch
    # gather batches group consecutive tiles
    batches = []
    t0 = 0
    while t0 < ntr:
        t1 = min(t0 + GB, ntr)
        batches.append((t0, t1))
        t0 = t1
    MAXB = max(blk_off[b1] - blk_off[b0] for b0, b1 in batches)

    nc = bacc.Bacc("TRN2", target_bir_lowering=False, debug=False, num_devices=R * C)
    dt = nc.dram_tensor
    # inputs
    t_dst = dt("dst_idx", [P, ECAP // 16], I16, kind="ExternalInput").ap()
    t_S = dt("S_mat", [P, ECAP], F8, kind="ExternalInput").ap()
    t_ST = dt("ST_mat", [P, ECAP], F8, kind="ExternalInput").ap()
    t_eb = dt("ebT", [P, NBLK * H], F16, kind="ExternalInput").ap()
    t_kT = dt("kT", [QD, DHALF], F16, kind="ExternalInput").ap()
    t_vT = dt("vT", [QD, DHALF], F16, kind="ExternalInput").ap()
    t_qT = dt("qT", [QD, NROW], F16, kind="ExternalInput").ap()
    t_WkT = dt("WkT", [QD, TD], F16, kind="ExternalInput").ap()
    t_WvT = dt("WvT", [QD, TD], F16, kind="ExternalInput").ap()
    t_WqT = dt("WqT", [QD, TD], F16, kind="ExternalInput").ap()
    t_WoT = dt("WoT", [TD, QD], F16, kind="ExternalInput").ap()
    t_id = dt("identity", [P, P], F16, kind="ExternalInput").ap()
    # internal DRAM (split nd tensors so Tile's tensor-granular dependency
    # tracking doesn't serialize later writes behind collectives)
    t_khv = dt("khv_tab", [DHALF, 2 * TD], F16).ap()
    t_nd_a = dt("nd_part_a", [H1 * P, 136], F16).ap()
    t_nd_b = dt("nd_part_b", [(ntr - H1) * P, 136], F16).ap()
    t_ndr_a = dt("nd_red_a", [HA * P, 136], F16).ap()
    t_ndr_b = dt("nd_red_b", [HB * P, 136], F16).ap()
    # output: core (i, j) finalizes tiles [j*HA, j*HA+HA) and
    # [H1 + j*HB, H1 + j*HB + HB) of its row
    t_out = dt("o_out", [(HA + HB) * P, QD], F16, kind="ExternalOutput").ap()

    dma_sem = nc.alloc_semaphore("swdge_dma")

    with tile.TileContext(nc) as tc:
        with (
            tc.tile_pool(name="const", bufs=1) as cpool,
            tc.tile_pool(name="proj", bufs=3) as ppool,
            tc.tile_pool(name="gath", bufs=3) as gpool,
            tc.tile_pool(name="work", bufs=2) as wpool,
            tc.tile_pool(name="out", bufs=2) as opool,
            tc.tile_pool(name="psA", bufs=2, space="PSUM") as psA,
            tc.tile_pool(name="psB", bufs=2, space="PSUM") as psB,
            tc.tile_pool(name="psC", bufs=2, space="PSUM") as psC,
        ):
            # ---- constants to SBUF ----
            c_WkT = cpool.tile([P, 2 * TD], F16)
            nc.sync.dma_start(out=c_WkT[:, 0:TD], in_=t_WkT[0:P, :])
            nc.sync.dma_start(out=c_WkT[:, TD:2 * TD], in_=t_WkT[P:QD, :])
            c_WvT = cpool.tile([P, 2 * TD], F16)
            nc.sync.dma_start(out=c_WvT[:, 0:TD], in_=t_WvT[0:P, :])
            nc.sync.dma_start(out=c_WvT[:, TD:2 * TD], in_=t_WvT[P:QD, :])
            c_WqT = cpool.tile([P, 2 * TD], F16)
            nc.sync.dma_start(out=c_WqT[:, 0:TD], in_=t_WqT[0:P, :])
            nc.sync.dma_start(out=c_WqT[:, TD:2 * TD], in_=t_WqT[P:QD, :])
            c_WoT = cpool.tile([TD, QD], F16); nc.sync.dma_start(out=c_WoT[:], in_=t_WoT)
            c_id = cpool.tile([P, P], F16); nc.sync.dma_start(out=c_id[:], in_=t_id)
            c_dsti = cpool.tile([P, ECAP // 16], I16)
            nc.sync.dma_start(out=c_dsti[:], in_=t_dst)
            qh_sb = cpool.tile([P, NQ * TD], F16)

            # ---- phase A: projections (qh -> SBUF table, khv -> HBM) ----
            def project_qh():
                for g0 in range(0, NQ, 8):
                    g1 = min(g0 + 8, NQ)
                    w = (g1 - g0) * P
                    ina = ppool.tile([P, 1024], F16, tag="ina")
                    inb = ppool.tile([P, 1024], F16, tag="inb")
                    nc.sync.dma_start(out=ina[:, :w], in_=t_qT[0:P, g0 * P:g0 * P + w])
                    nc.sync.dma_start(out=inb[:, :w], in_=t_qT[P:QD, g0 * P:g0 * P + w])
                    for cc in range(g0, g1):
                        o = (cc - g0) * P
                        ps = psA.tile([P, 4, TD], F32, tag="psq")
                        nc.tensor.matmul(out=ps[:, 0, :], lhsT=ina[:, o:o + P],
                                         rhs=c_WqT[:, 0:TD], start=True, stop=False)
                        nc.tensor.matmul(out=ps[:, 0, :], lhsT=inb[:, o:o + P],
                                         rhs=c_WqT[:, TD:2 * TD], start=False, stop=True)
                        nc.scalar.activation(out=qh_sb[:, cc * TD:(cc + 1) * TD],
                                             in_=ps[:, 0, :], func=AF.Copy)

            # khv table: interleave kh|vh per node row; each core projects the
            # whole dst half locally, straight into t_khv.
            def project_khv(r0, r1):
                for g0 in range(r0 // P, r1 // P, 8):
                    g1 = min(g0 + 8, r1 // P)
                    w = (g1 - g0) * P
                    ka = ppool.tile([P, 1024], F16, tag="ka")
                    kb = ppool.tile([P, 1024], F16, tag="kb")
                    va = ppool.tile([P, 1024], F16, tag="va")
                    vb = ppool.tile([P, 1024], F16, tag="vb")
                    nc.sync.dma_start(out=ka[:, :w], in_=t_kT[0:P, g0 * P:g0 * P + w])
                    nc.sync.dma_start(out=kb[:, :w], in_=t_kT[P:QD, g0 * P:g0 * P + w])
                    nc.sync.dma_start(out=va[:, :w], in_=t_vT[0:P, g0 * P:g0 * P + w])
                    nc.sync.dma_start(out=vb[:, :w], in_=t_vT[P:QD, g0 * P:g0 * P + w])
                    stage = ppool.tile([P, 8 * 2 * TD], F16, tag="kvstage")
                    for cc in range(g0, g1):
                        o = (cc - g0) * P
                        ps = psA.tile([P, 4, TD], F32, tag="psq")
                        nc.tensor.matmul(out=ps[:, 0, :], lhsT=ka[:, o:o + P],
                                         rhs=c_WkT[:, 0:TD], start=True, stop=False)
                        nc.tensor.matmul(out=ps[:, 0, :], lhsT=kb[:, o:o + P],
                                         rhs=c_WkT[:, TD:2 * TD], start=False, stop=True)
                        nc.tensor.matmul(out=ps[:, 1, :], lhsT=va[:, o:o + P],
                                         rhs=c_WvT[:, 0:TD], start=True, stop=False)
                        nc.tensor.matmul(out=ps[:, 1, :], lhsT=vb[:, o:o + P],
                                         rhs=c_WvT[:, TD:2 * TD], start=False, stop=True)
                        nc.scalar.activation(
                            out=stage[:, (cc - g0) * 256:(cc - g0) * 256 + 256],
                            in_=ps[:, 0:2, :].rearrange("p b f -> p (b f)"),
                            func=AF.Copy)
                    nc.sync.dma_start(
                        out=t_khv[g0 * P:g1 * P, :].rearrange("(c p) w -> p c w", p=P),
                        in_=stage[:, :(g1 - g0) * 256].rearrange(
                            "p (c w) -> p c w", w=256))

            project_khv(0, DHALF)
            project_qh()

            pairs = [[0, 1], [2, 3], [4, 5], [6, 7]]

            def reduce_scatter(t_in, t_out_):
                with nc.allow_low_precision(reason="f16 num/den partials"):
                    nc.gpsimd.collective_compute(
                        "ReduceScatter", ALU.add, replica_groups=pairs,
                        ins=[t_in], outs=[t_out_])

            # ---- finalize: normalize + output projection for my tiles ----
            def finalize(t, src_nd, out_tile):
                ndl = opool.tile([P, 136], F16, tag="ndl")
                nc.sync.dma_start(out=ndl[:], in_=src_nd[t * P:(t + 1) * P, :])
                rden = opool.tile([P, H], F32, tag="rden")
                # +eps in f32 so empty node slots yield 0 instead of 0*inf=NaN
                # (a NaN row would poison the whole tile through the PE transpose)
                nc.vector.tensor_scalar_add(out=rden[:], in0=ndl[:, TD:TD + H],
                                            scalar1=1e-30)
                nc.vector.reciprocal(out=rden[:], in_=rden[:])
                o_sb = opool.tile([P, TD], F16, tag="o_sb")
                with nc.allow_low_precision(reason="f16 normalized output"):
                    nc.vector.tensor_tensor(
                        out=o_sb[:].rearrange("p (h d) -> p h d", h=H),
                        in0=ndl[:, 0:TD].rearrange("p (h d) -> p h d", h=H),
                        in1=rden[:, :, None].to_broadcast([P, H, D]),
                        op=ALU.mult)
                ps_oT = psC.tile([P, P], F16, tag="oT")
                nc.tensor.transpose(out=ps_oT[:], in_=o_sb[:], identity=c_id[:])
                oT_sb = opool.tile([P, P], F16, tag="oT_sb")
                nc.scalar.activation(out=oT_sb[:], in_=ps_oT[:], func=AF.Copy)
                ps_o = psC.tile([P, QD], F32, tag="ps_o")
                nc.tensor.matmul(out=ps_o[:], lhsT=oT_sb[:], rhs=c_WoT[:],
                                 start=True, stop=True)
                out_sb = opool.tile([P, QD], F16, tag="out_sb")
                with nc.allow_low_precision(reason="f16 output"):
                    nc.scalar.activation(out=out_sb[:], in_=ps_o[:], func=AF.Copy)
                nc.sync.dma_start(out=t_out[out_tile * P:(out_tile + 1) * P, :],
                                  in_=out_sb[:])

            # ---- phase M: main loop over gather batches ----
            n_subg = [0]

            for (b0, b1) in batches:
                e0, e1 = blk_off[b0] * P, blk_off[b1] * P
                ne = e1 - e0
                nbb = ne // P
                khv_g = gpool.tile([P, MAXB, 2 * TD], F16, tag="khv_g")
                # prepare_only + trigger_dma: descriptor generation (the Q7
                # bottleneck, ~8ns/edge) pipelines with the transfers
                nc.gpsimd.dma_gather(
                    out_ap=khv_g[:, :nbb, :], in_ap=t_khv,
                    idxs_ap=c_dsti[:, e0 // 16:e1 // 16],
                    num_idxs=ne, num_idxs_reg=ne, elem_size=2 * TD,
                    single_packet=False, prepare_only=True, sem=dma_sem)
                nc.gpsimd.trigger_dma(count=None)
                n_subg[0] += 1
                wait_thresh = 16 * n_subg[0]
                S_sb = gpool.tile([P, MAXB * P], F8, tag="S_sb")
                nc.sync.dma_start(out=S_sb[:, :ne], in_=t_S[:, e0:e1])
                ST_sb = gpool.tile([P, MAXB * P], F8, tag="ST_sb")
                nc.sync.dma_start(out=ST_sb[:, :ne], in_=t_ST[:, e0:e1])
                eb_sb = gpool.tile([P, MAXB * H], F16, tag="eb_sb")
                nc.sync.dma_start(out=eb_sb[:, :nbb * H],
                                  in_=t_eb[:, blk_off[b0] * H:blk_off[b1] * H])

                # qh rows via one-hot matmuls, copied to f16 working buffer
                qsb = wpool.tile([P, MAXB, TD], F16, tag="qsb")
                for t in range(b0, b1):
                    nb = bpt[t]
                    go = blk_off[t] - blk_off[b0]    # block offset in batch
                    for g in range(0, nb, 4):
                        gn = min(4, nb - g)
                        ps_q = psA.tile([P, 4, TD], F32, tag="psq")
                        for b in range(g, g + gn):
                            nc.tensor.matmul(
                                out=ps_q[:, b - g, :],
                                lhsT=ST_sb[:, (go + b) * P:(go + b + 1) * P],
                                rhs=qh_sb[:, t * TD:(t + 1) * TD],
                                start=True, stop=True)
                        with nc.allow_low_precision(reason="f16 qh rows"):
                            nc.scalar.activation(
                                out=qsb[:, go + g:go + g + gn, :].rearrange("p b f -> p (b f)"),
                                in_=ps_q[:, 0:gn, :].rearrange("p b f -> p (b f)"),
                                func=AF.Copy)

                # batched logits: prod = qh*kh, qk = sum_d, attn = qk + eb
                # (explicit wait: gather-DMA completion is NOT implied by the
                # prep's engine tick, so the first khv_g consumer must wait on
                # the descriptor-baked DMA semaphore)
                nc.vector.wait_ge(dma_sem, wait_thresh)
                prod = wpool.tile([P, MAXB, TD], F16, tag="prod")
                nc.vector.tensor_tensor(
                    out=prod[:, :nbb, :], in0=qsb[:, :nbb, :],
                    in1=khv_g[:, :nbb, 0:TD], op=ALU.mult)
                qk = wpool.tile([P, MAXB, H], F16, tag="qk")
                with nc.allow_low_precision(reason="f16 qk logits"):
                    nc.vector.reduce_sum(
                        out=qk[:, :nbb, :],
                        in_=prod[:, :nbb, :].rearrange("p b (h d) -> p b h d", h=H),
                        axis=mybir.AxisListType.X)
                attn = wpool.tile([P, MAXB * H], F16, tag="attn")
                with nc.allow_low_precision(reason="f16 logits"):
                    nc.vector.tensor_tensor(
                        out=attn[:, :nbb * H],
                        in0=qk[:, :nbb, :].rearrange("p b h -> p (b h)"),
                        in1=eb_sb[:, :nbb * H], op=ALU.add)
                w_t = wpool.tile([P, MAXB, H], F16, tag="w")
                with nc.allow_low_precision(reason="f16 weights"):
                    nc.scalar.activation(out=w_t[:, :nbb, :].rearrange("p b h -> p (b h)"),
                                         in_=attn[:, :nbb * H], func=AF.Exp)
                # rhs = [w*vh | w]
                wv = wpool.tile([P, MAXB, 136], F16, tag="wv")
                with nc.allow_low_precision(reason="f16 weighted values"):
                    nc.vector.tensor_tensor(
                        out=wv[:, :nbb, 0:TD].rearrange("p b (h d) -> p b h d", h=H),
                        in0=khv_g[:, :nbb, TD:2 * TD].rearrange("p b (h d) -> p b h d", h=H),
                        in1=w_t[:, :nbb, :, None].to_broadcast([P, nbb, H, D]),
                        op=ALU.mult)
                    nc.vector.tensor_copy(out=wv[:, :nbb, TD:TD + H], in_=w_t[:, :nbb, :])

                # scatter-accumulate into num|den psum per tile
                for t in range(b0, b1):
                    nb = bpt[t]
                    go = blk_off[t] - blk_off[b0]
                    ps_nd = psB.tile([P, 136], F32, tag="nd")
                    for b in range(nb):
                        nc.tensor.matmul(out=ps_nd[:],
                                         lhsT=S_sb[:, (go + b) * P:(go + b + 1) * P],
                                         rhs=wv[:, go + b, :],
                                         start=(b == 0), stop=(b == nb - 1))
                    nd_sb = opool.tile([P, 136], F16, tag="nd_sb")
                    with nc.allow_low_precision(reason="f16 num/den partials"):
                        nc.scalar.activation(out=nd_sb[:], in_=ps_nd[:], func=AF.Copy)
                    if t < H1:
                        nc.sync.dma_start(out=t_nd_a[t * P:(t + 1) * P, :], in_=nd_sb[:])
                    else:
                        nc.sync.dma_start(
                            out=t_nd_b[(t - H1) * P:(t - H1 + 1) * P, :], in_=nd_sb[:])

            # ---- tail: reduce-scatter + finalize (post-loop so the cc ops
            # never block the gpsimd descriptor-generation stream) ----
            reduce_scatter(t_nd_a, t_ndr_a)
            reduce_scatter(t_nd_b, t_ndr_b)
            for tf in range(HA):
                finalize(tf, t_ndr_a, tf)
            for t in range(HB):
                finalize(t, t_ndr_b, HA + t)

    nc.compile()
    return nc


_CACHE = {}
LAST_RUN = {}


def kernel(**inputs) -> np.ndarray:
    q = np.asarray(inputs["q"], np.float32)
    k = np.asarray(inputs["k"], np.float32)
    v = np.asarray(inputs["v"], np.float32)
    edges = np.asarray(inputs["edges"], np.float32)
    edge_index = np.asarray(inputs["edge_index"])
    Wq, Wk, Wv = inputs["Wq"], inputs["Wk"], inputs["Wv"]
    Wb, bb, Wo, bo = inputs["Wb"], inputs["bb"], inputs["Wo"], inputs["bo"]

    cores, consts, meta = prepare(q, k, v, edges, edge_index, Wq, Wk, Wv, Wb, bb, Wo, bo)
    N = meta["N"]
    ntr = meta["ntiles_row"]
    H1 = ((ntr // 2 + 1) // 2) * 2
    HA, HB = H1 // 2, (ntr - H1) // 2

    key = (q.shape, edges.shape, meta["NBLK"])
    if key not in _CACHE:
        _CACHE[key] = build_program(meta)
    nc = _CACHE[key]

    in_maps = []
    for core in range(R * C):
        m = dict(cores[core])
        m.update({kk: np.ascontiguousarray(vv) for kk, vv in consts.items()})
        in_maps.append({kk: np.ascontiguousarray(vv) for kk, vv in m.items()})

    import os
    if os.environ.get("KERNEL_SIM"):
        from concourse.bass_interp import MultiCoreSim
        sim = MultiCoreSim(nc, num_cores=R * C)
        for ci, core_sim in sim.cores.items():
            for name, arr in in_maps[ci].items():
                core_sim.tensor(name)[:] = arr
        sim.simulate(check_with_hw=False)
        results = [{"o_out": np.array(sim.cores[ci].tensor("o_out"))}
                   for ci in range(R * C)]
    else:
        trace = bool(os.environ.get("KERNEL_TRACE"))
        res = bass_utils.run_bass_kernel_spmd(nc, in_maps, core_ids=list(range(R * C)),
                                              trace=trace)
        LAST_RUN["res"] = res
        results = res.results

    # assemble: core (i, j) has tiles [j*HA, j*HA+HA) + [H1+j*HB, H1+j*HB+HB)
    out = np.zeros((meta["NPAD"], QD), np.float32)
    node_tile, node_slot = meta["node_tile"], meta["node_slot"]
    for i in range(R):
        for j in range(C):
            o = results[i * C + j]["o_out"].astype(np.float32)  # [(HA+HB)*P, QD]
            ta = j * HA
            out[(i * ntr + ta) * P:(i * ntr + ta + HA) * P] = o[0:HA * P]
            tb = H1 + j * HB
            out[(i * ntr + tb) * P:(i * ntr + tb + HB) * P] = o[HA * P:(HA + HB) * P]
    # map back to node ids
    full = np.zeros((N, QD), np.float32)
    rowpos = node_tile * P + node_slot
    full[:, :] = out[rowpos[np.arange(N)]]
    full += np.asarray(bo, np.float32)[None, :]
    # zero-degree nodes: reference yields bo
    zd = meta["deg"] == 0
    if zd.any():
        full[zd] = np.asarray(bo, np.float32)[None, :]
    return full


# revision 42
# speedup vs baseline: 1.1016x; 1.1016x over previous
"""GNN edge-softmax attention kernel for 8 Trainium2 NeuronCores.

Strategy (4 src-rows x 2 dst-halves core grid):
  - Host routes each edge to core (row(src), half(dst)). Nodes are packed
    into 128-node tiles balanced by edge count; each tile's edges are padded
    to whole 128-edge blocks so every core runs an identical program.
  - Host precomputes the edge-bias term eb = edges @ Wb.T + bb (tiny linear
    map) so the device never touches the raw 64-dim edge features.
  - Per core: project q/k/v slices with PE into f16 tables (khv table
    assembled across the quad with a 2-chunk AllGather overlapped with the
    qh projection), then for each gather batch: pipelined dma_gather
    (prepare_only + trigger_dma, so descriptor prep never blocks on DMA),
    per-tile one-hot matmuls for qh rows, batched DVE mult/reduce for the
    per-edge logits, exp on ACT, and scatter-accumulate num/den into PSUM
    with selection-matrix matmuls.
  - num/den partials are written in f16 and pair-ReduceScattered in two
    chunks (first chunk overlaps the main loop); each core normalizes and
    applies the output projection for its half of the tiles. Host adds bo.
"""

import math
import sys

import numpy as np

sys.path.insert(0, "/opt/trn_rl_repo")

import concourse.bacc as bacc
import concourse.bass as bass
import concourse.mybir as mybir
import concourse.tile as tile
from concourse import bass_utils

F16 = mybir.dt.float16
F8 = mybir.dt.float8e4
F32 = mybir.dt.float32
I16 = mybir.dt.int16

H = 8            # heads
D = 16           # head dim
TD = H * D       # 128
QD = 256         # q/k/v feature dim
PD = 64          # edge pair feature dim
R = 4            # src rows of the core grid
C = 2            # dst cols of the core grid
P = 128

AF = mybir.ActivationFunctionType
ALU = mybir.AluOpType


def _wrap16(idx: np.ndarray) -> np.ndarray:
    """dma_gather index layout: [128, n/16] with idx i at (i%16 + 16k, i//16)."""
    n = idx.shape[0]
    assert n % 16 == 0
    w = idx.reshape(n // 16, 16).T.astype(np.int16)  # [16, n/16]
    return np.tile(w, (8, 1))  # replicate across the 8 partition groups


def prepare(q, k, v, edges, edge_index, Wq, Wk, Wv, Wb, bb, Wo, bo):
    N = q.shape[0]
    E = edges.shape[0]
    ntiles_row = math.ceil(N / (R * P))          # tiles per src row
    NROW = ntiles_row * P                        # nodes per row (padded)
    NPAD = NROW * R
    DHALF = NPAD // 2                            # dst-half size
    assert DHALF < 32768, "dst half must fit int16"
    Q4 = DHALF // 4                              # rows projected per core
    NQ4 = Q4 // P
    CH0 = ((NQ4 + 1) // 2) * P                   # chunk-0 rows (per member)
    CH1 = Q4 - CH0                               # chunk-1 rows

    src = np.asarray(edge_index[:, 0], dtype=np.int64)
    dst = np.asarray(edge_index[:, 1], dtype=np.int64)
    deg = np.bincount(src, minlength=N)

    # --- greedy node->tile packing balanced by edge count ---
    T = R * ntiles_row
    order = np.argsort(-deg, kind="stable")
    tile_cnt = np.zeros(T, dtype=np.int64)       # nodes in tile
    tile_edges = np.zeros(T, dtype=np.int64)
    node_tile = np.zeros(N, dtype=np.int32)
    node_slot = np.zeros(N, dtype=np.int32)
    import heapq
    heap = [(0, t) for t in range(T)]
    heapq.heapify(heap)
    for n in order:
        while True:
            e_cnt, t = heapq.heappop(heap)
            if tile_cnt[t] < P:
                break
        node_tile[n] = t
        node_slot[n] = tile_cnt[t]
        tile_cnt[t] += 1
        tile_edges[t] += deg[n]
        if tile_cnt[t] < P:
            heapq.heappush(heap, (tile_edges[t], t))

    row_of_edge = node_tile[src] // ntiles_row
    j_of_edge = (dst // DHALF).astype(np.int64)
    tloc_of_edge = (node_tile[src] % ntiles_row).astype(np.int64)

    # per (core, tile_local) edge counts -> shared block counts per tile slot
    core_of_edge = row_of_edge * C + j_of_edge
    cnt = np.zeros((R * C, ntiles_row), dtype=np.int64)
    np.add.at(cnt, (core_of_edge, tloc_of_edge), 1)
    bpt = np.maximum(1, np.ceil(cnt.max(axis=0) / P).astype(np.int64))  # [ntiles_row]
    blk_off = np.concatenate([[0], np.cumsum(bpt)])   # block offset per tile
    NBLK = int(blk_off[-1])
    ECAP = NBLK * P

    # host-side edge bias: eb[e, h] = edges @ Wb.T + bb
    eb_all = (np.asarray(edges, np.float32) @ np.asarray(Wb, np.float32).T
              + np.asarray(bb, np.float32)[None, :]).astype(np.float16)  # [E, H]

    # khv table row for a dst node local to half j (chunk-major layout so the
    # AllGather can run in two chunks writing contiguous table regions)
    def table_row(loc):
        m = loc // Q4
        r = loc % Q4
        return np.where(r < CH0, m * CH0 + r, 4 * CH0 + m * CH1 + (r - CH0))

    # --- per-core edge arrays ---
    cores = []
    for core in range(R * C):
        i, j = core // C, core % C
        mask = core_of_edge == core
        es, ed, et = src[mask], dst[mask], tloc_of_edge[mask]
        # order edges by tile slot
        ordr = np.argsort(et, kind="stable")
        es, ed, et = es[ordr], ed[ordr], et[ordr]
        # positions: per tile, fill from blk_off[t]*P
        pos = np.zeros(len(es), dtype=np.int64)
        start = 0
        for t in range(ntiles_row):
            c = int((et == t).sum())
            pos[start:start + c] = blk_off[t] * P + np.arange(c)
            start += c
        eidx = np.nonzero(mask)[0][ordr]

        import ml_dtypes
        F8NP = ml_dtypes.float8_e4m3
        dst_local = np.zeros(ECAP, dtype=np.int16)
        src_rel = np.full(ECAP, 255, dtype=np.int64)
        ebE = np.zeros((ECAP, H), dtype=np.float16)
        dst_local[pos] = table_row(ed - j * DHALF).astype(np.int16)
        src_rel[pos] = node_slot[es]
        ebE[pos] = eb_all[eidx]
        # eb in edge-major block layout [128, NBLK*H]
        ebT = np.ascontiguousarray(
            ebE.reshape(NBLK, P, H).transpose(1, 0, 2)).reshape(P, NBLK * H)
        # one-hot selection matrices (fp8, exact 0/1)
        S_en = np.zeros((ECAP, P), dtype=F8NP)
        valid = src_rel < P
        S_en[np.nonzero(valid)[0], src_rel[valid]] = 1.0
        S_en3 = S_en.reshape(NBLK, P, P)                       # [b, e, n]
        S_mat = np.ascontiguousarray(S_en3.transpose(1, 0, 2)).reshape(P, ECAP)   # [e_part, (b n)]
        ST_mat = np.ascontiguousarray(S_en3.transpose(2, 0, 1)).reshape(P, ECAP)  # [n_part, (b e)]

        # constants: this core projects quarter i of half j's khv table
        qlo = j * DHALF + i * Q4
        qhi = min(qlo + Q4, N)
        kT = np.zeros((QD, Q4), dtype=np.float16)
        vT = np.zeros((QD, Q4), dtype=np.float16)
        if qhi > qlo:
            kT[:, :qhi - qlo] = np.asarray(k[qlo:qhi], np.float32).T.astype(np.float16)
            vT[:, :qhi - qlo] = np.asarray(v[qlo:qhi], np.float32).T.astype(np.float16)
        # q rows permuted into (tile_local, slot) order for this row i
        qT = np.zeros((QD, NROW), dtype=np.float16)
        rmask = node_tile // ntiles_row == i
        rn = np.nonzero(rmask)[0]
        qpos = (node_tile[rn] % ntiles_row) * P + node_slot[rn]
        qT[:, qpos] = np.asarray(q[rn], np.float32).T.astype(np.float16)

        cores.append(dict(
            dst_idx=_wrap16(dst_local), S_mat=S_mat, ST_mat=ST_mat,
            ebT=ebT, kT=kT, vT=vT, qT=qT,
        ))

    norm = D ** -0.5
    consts = dict(
        WkT=np.asarray(Wk, np.float32).T.astype(np.float16),
        WvT=np.asarray(Wv, np.float32).T.astype(np.float16),
        WqT=(np.asarray(Wq, np.float32) * norm).T.astype(np.float16),
        WoT=np.asarray(Wo, np.float32).T.astype(np.float16),
        identity=np.eye(P, dtype=np.float16),
    )
    meta = dict(N=N, NPAD=NPAD, NROW=NROW, DHALF=DHALF, ntiles_row=ntiles_row,
                NBLK=NBLK, ECAP=ECAP, bpt=bpt.tolist(), blk_off=blk_off.tolist(),
                CH0=CH0, CH1=CH1,
                node_tile=node_tile, node_slot=node_slot, deg=deg)
    return cores, consts, meta


def build_program(meta, gather_batch=3):
    """Build the SPMD bass program. Returns compiled nc."""
    ntr = meta["ntiles_row"]
    NROW, DHALF = meta["NROW"], meta["DHALF"]
    NBLK, ECAP = meta["NBLK"], meta["ECAP"]
    bpt, blk_off = meta["bpt"], meta["blk_off"]
    CH0, CH1 = meta["CH0"], meta["CH1"]
    Q4 = DHALF // 4
    NQ = NROW // P        # qh chunks
    # ReduceScatter split: chunk A covers tiles [0, H1), chunk B the rest.
    H1 = ((ntr // 2 + 1) // 2) * 2  # even tile count near the middle
    HA, HB = H1 // 2, (ntr - H1) // 2
    assert H1 % 2 == 0 and (ntr - H1) % 2 == 0

    GB = gather_batch
    # gather batches group consecutive tiles
    batches = []
    t0 = 0
    while t0 < ntr:
        t1 = min(t0 + GB, ntr)
        batches.append((t0, t1))
        t0 = t1
    MAXB = max(blk_off[b1] - blk_off[b0] for b0, b1 in batches)

    nc = bacc.Bacc("TRN2", target_bir_lowering=False, debug=False, num_devices=R * C)
    dt = nc.dram_tensor
    # inputs
    t_dst = dt("dst_idx", [P, ECAP // 16], I16, kind="ExternalInput").ap()
    t_S = dt("S_mat", [P, ECAP], F8, kind="ExternalInput").ap()
    t_ST = dt("ST_mat", [P, ECAP], F8, kind="ExternalInput").ap()
    t_eb = dt("ebT", [P, NBLK * H], F16, kind="ExternalInput").ap()
    t_kT = dt("kT", [QD, Q4], F16, kind="ExternalInput").ap()
    t_vT = dt("vT", [QD, Q4], F16, kind="ExternalInput").ap()
    t_qT = dt("qT", [QD, NROW], F16, kind="ExternalInput").ap()
    t_WkT = dt("WkT", [QD, TD], F16, kind="ExternalInput").ap()
    t_WvT = dt("WvT", [QD, TD], F16, kind="ExternalInput").ap()
    t_WqT = dt("WqT", [QD, TD], F16, kind="ExternalInput").ap()
    t_WoT = dt("WoT", [TD, QD], F16, kind="ExternalInput").ap()
    t_id = dt("identity", [P, P], F16, kind="ExternalInput").ap()
    # internal DRAM (chunked tensors are separate so Tile's tensor-granular
    # dependency tracking doesn't serialize later writes behind collectives)
    t_khv = dt("khv_tab", [DHALF, 2 * TD], F16).ap()
    t_khv_h0 = dt("khv_half0", [CH0, 2 * TD], F16).ap()
    t_khv_h1 = dt("khv_half1", [CH1, 2 * TD], F16).ap()
    t_nd_a = dt("nd_part_a", [H1 * P, 136], F16).ap()
    t_nd_b = dt("nd_part_b", [(ntr - H1) * P, 136], F16).ap()
    t_ndr_a = dt("nd_red_a", [HA * P, 136], F16).ap()
    t_ndr_b = dt("nd_red_b", [HB * P, 136], F16).ap()
    # output: core (i, j) finalizes tiles [j*HA, j*HA+HA) and
    # [H1 + j*HB, H1 + j*HB + HB) of its row
    t_out = dt("o_out", [(HA + HB) * P, QD], F16, kind="ExternalOutput").ap()

    dma_sem = nc.alloc_semaphore("swdge_dma")

    with tile.TileContext(nc) as tc:
        with (
            tc.tile_pool(name="const", bufs=1) as cpool,
            tc.tile_pool(name="proj", bufs=3) as ppool,
            tc.tile_pool(name="gath", bufs=3) as gpool,
            tc.tile_pool(name="work", bufs=2) as wpool,
            tc.tile_pool(name="out", bufs=2) as opool,
            tc.tile_pool(name="psA", bufs=2, space="PSUM") as psA,
            tc.tile_pool(name="psB", bufs=2, space="PSUM") as psB,
            tc.tile_pool(name="psC", bufs=2, space="PSUM") as psC,
        ):
            # ---- constants to SBUF ----
            c_WkT = cpool.tile([P, 2 * TD], F16)
            nc.sync.dma_start(out=c_WkT[:, 0:TD], in_=t_WkT[0:P, :])
            nc.sync.dma_start(out=c_WkT[:, TD:2 * TD], in_=t_WkT[P:QD, :])
            c_WvT = cpool.tile([P, 2 * TD], F16)
            nc.sync.dma_start(out=c_WvT[:, 0:TD], in_=t_WvT[0:P, :])
            nc.sync.dma_start(out=c_WvT[:, TD:2 * TD], in_=t_WvT[P:QD, :])
            c_WqT = cpool.tile([P, 2 * TD], F16)
            nc.sync.dma_start(out=c_WqT[:, 0:TD], in_=t_WqT[0:P, :])
            nc.sync.dma_start(out=c_WqT[:, TD:2 * TD], in_=t_WqT[P:QD, :])
            c_WoT = cpool.tile([TD, QD], F16); nc.sync.dma_start(out=c_WoT[:], in_=t_WoT)
            c_id = cpool.tile([P, P], F16); nc.sync.dma_start(out=c_id[:], in_=t_id)
            c_dsti = cpool.tile([P, ECAP // 16], I16)
            nc.sync.dma_start(out=c_dsti[:], in_=t_dst)
            qh_sb = cpool.tile([P, NQ * TD], F16)

            # ---- phase A: projections (qh -> SBUF table, khv -> HBM) ----
            def project_qh():
                for g0 in range(0, NQ, 8):
                    g1 = min(g0 + 8, NQ)
                    w = (g1 - g0) * P
                    ina = ppool.tile([P, 1024], F16, tag="ina")
                    inb = ppool.tile([P, 1024], F16, tag="inb")
                    nc.sync.dma_start(out=ina[:, :w], in_=t_qT[0:P, g0 * P:g0 * P + w])
                    nc.sync.dma_start(out=inb[:, :w], in_=t_qT[P:QD, g0 * P:g0 * P + w])
                    for cc in range(g0, g1):
                        o = (cc - g0) * P
                        ps = psA.tile([P, 4, TD], F32, tag="psq")
                        nc.tensor.matmul(out=ps[:, 0, :], lhsT=ina[:, o:o + P],
                                         rhs=c_WqT[:, 0:TD], start=True, stop=False)
                        nc.tensor.matmul(out=ps[:, 0, :], lhsT=inb[:, o:o + P],
                                         rhs=c_WqT[:, TD:2 * TD], start=False, stop=True)
                        nc.scalar.activation(out=qh_sb[:, cc * TD:(cc + 1) * TD],
                                             in_=ps[:, 0, :], func=AF.Copy)

            # khv table: interleave kh|vh per node row; each core projects its
            # quarter of the half, 2-chunk AllGather assembles t_khv.
            def project_khv(r0, r1, t_half):
                for g0 in range(r0 // P, r1 // P, 8):
                    g1 = min(g0 + 8, r1 // P)
                    w = (g1 - g0) * P
                    ka = ppool.tile([P, 1024], F16, tag="ka")
                    kb = ppool.tile([P, 1024], F16, tag="kb")
                    va = ppool.tile([P, 1024], F16, tag="va")
                    vb = ppool.tile([P, 1024], F16, tag="vb")
                    nc.sync.dma_start(out=ka[:, :w], in_=t_kT[0:P, g0 * P:g0 * P + w])
                    nc.sync.dma_start(out=kb[:, :w], in_=t_kT[P:QD, g0 * P:g0 * P + w])
                    nc.sync.dma_start(out=va[:, :w], in_=t_vT[0:P, g0 * P:g0 * P + w])
                    nc.sync.dma_start(out=vb[:, :w], in_=t_vT[P:QD, g0 * P:g0 * P + w])
                    stage = ppool.tile([P, 8 * 2 * TD], F16, tag="kvstage")
                    for cc in range(g0, g1):
                        o = (cc - g0) * P
                        ps = psA.tile([P, 4, TD], F32, tag="psq")
                        nc.tensor.matmul(out=ps[:, 0, :], lhsT=ka[:, o:o + P],
                                         rhs=c_WkT[:, 0:TD], start=True, stop=False)
                        nc.tensor.matmul(out=ps[:, 0, :], lhsT=kb[:, o:o + P],
                                         rhs=c_WkT[:, TD:2 * TD], start=False, stop=True)
                        nc.tensor.matmul(out=ps[:, 1, :], lhsT=va[:, o:o + P],
                                         rhs=c_WvT[:, 0:TD], start=True, stop=False)
                        nc.tensor.matmul(out=ps[:, 1, :], lhsT=vb[:, o:o + P],
                                         rhs=c_WvT[:, TD:2 * TD], start=False, stop=True)
                        nc.scalar.activation(
                            out=stage[:, (cc - g0) * 256:(cc - g0) * 256 + 256],
                            in_=ps[:, 0:2, :].rearrange("p b f -> p (b f)"),
                            func=AF.Copy)
                    nc.sync.dma_start(
                        out=t_half[(g0 - r0 // P) * P:(g1 - r0 // P) * P, :].rearrange(
                            "(c p) w -> p c w", p=P),
                        in_=stage[:, :(g1 - g0) * 256].rearrange(
                            "p (c w) -> p c w", w=256))

            quads = [[0, 2, 4, 6], [1, 3, 5, 7]]
            project_khv(0, CH0, t_khv_h0)
            nc.gpsimd.collective_compute(
                "AllGather", ALU.bypass, replica_groups=quads,
                ins=[t_khv_h0], outs=[t_khv[0:4 * CH0, :]])
            project_khv(CH0, Q4, t_khv_h1)
            nc.gpsimd.collective_compute(
                "AllGather", ALU.bypass, replica_groups=quads,
                ins=[t_khv_h1], outs=[t_khv[4 * CH0:DHALF, :]])
            project_qh()

            pairs = [[0, 1], [2, 3], [4, 5], [6, 7]]

            def reduce_scatter(t_in, t_out_):
                with nc.allow_low_precision(reason="f16 num/den partials"):
                    nc.gpsimd.collective_compute(
                        "ReduceScatter", ALU.add, replica_groups=pairs,
                        ins=[t_in], outs=[t_out_])

            # ---- finalize: normalize + output projection for my tiles ----
            def finalize(t, src_nd, out_tile):
                ndl = opool.tile([P, 136], F16, tag="ndl")
                nc.sync.dma_start(out=ndl[:], in_=src_nd[t * P:(t + 1) * P, :])
                rden = opool.tile([P, H], F32, tag="rden")
                # +eps in f32 so empty node slots yield 0 instead of 0*inf=NaN
                # (a NaN row would poison the whole tile through the PE transpose)
                nc.vector.tensor_scalar_add(out=rden[:], in0=ndl[:, TD:TD + H],
                                            scalar1=1e-30)
                nc.vector.reciprocal(out=rden[:], in_=rden[:])
                o_sb = opool.tile([P, TD], F16, tag="o_sb")
                with nc.allow_low_precision(reason="f16 normalized output"):
                    nc.vector.tensor_tensor(
                        out=o_sb[:].rearrange("p (h d) -> p h d", h=H),
                        in0=ndl[:, 0:TD].rearrange("p (h d) -> p h d", h=H),
                        in1=rden[:, :, None].to_broadcast([P, H, D]),
                        op=ALU.mult)
                ps_oT = psC.tile([P, P], F16, tag="oT")
                nc.tensor.transpose(out=ps_oT[:], in_=o_sb[:], identity=c_id[:])
                oT_sb = opool.tile([P, P], F16, tag="oT_sb")
                nc.scalar.activation(out=oT_sb[:], in_=ps_oT[:], func=AF.Copy)
                ps_o = psC.tile([P, QD], F32, tag="ps_o")
                nc.tensor.matmul(out=ps_o[:], lhsT=oT_sb[:], rhs=c_WoT[:],
                                 start=True, stop=True)
                out_sb = opool.tile([P, QD], F16, tag="out_sb")
                with nc.allow_low_precision(reason="f16 output"):
                    nc.scalar.activation(out=out_sb[:], in_=ps_o[:], func=AF.Copy)
                nc.sync.dma_start(out=t_out[out_tile * P:(out_tile + 1) * P, :],
                                  in_=out_sb[:])

            # ---- phase M: main loop over gather batches ----
            n_subg = [0]

            for (b0, b1) in batches:
                e0, e1 = blk_off[b0] * P, blk_off[b1] * P
                ne = e1 - e0
                nbb = ne // P
                khv_g = gpool.tile([P, MAXB, 2 * TD], F16, tag="khv_g")
                # prepare_only + trigger_dma: descriptor generation (the Q7
                # bottleneck, ~8ns/edge) pipelines with the transfers
                nc.gpsimd.dma_gather(
                    out_ap=khv_g[:, :nbb, :], in_ap=t_khv,
                    idxs_ap=c_dsti[:, e0 // 16:e1 // 16],
                    num_idxs=ne, num_idxs_reg=ne, elem_size=2 * TD,
                    single_packet=False, prepare_only=True, sem=dma_sem)
                nc.gpsimd.trigger_dma(count=None)
                n_subg[0] += 1
                wait_thresh = 16 * n_subg[0]
                S_sb = gpool.tile([P, MAXB * P], F8, tag="S_sb")
                nc.sync.dma_start(out=S_sb[:, :ne], in_=t_S[:, e0:e1])
                ST_sb = gpool.tile([P, MAXB * P], F8, tag="ST_sb")
                nc.sync.dma_start(out=ST_sb[:, :ne], in_=t_ST[:, e0:e1])
                eb_sb = gpool.tile([P, MAXB * H], F16, tag="eb_sb")
                nc.sync.dma_start(out=eb_sb[:, :nbb * H],
                                  in_=t_eb[:, blk_off[b0] * H:blk_off[b1] * H])

                # qh rows via one-hot matmuls, copied to f16 working buffer
                qsb = wpool.tile([P, MAXB, TD], F16, tag="qsb")
                for t in range(b0, b1):
                    nb = bpt[t]
                    go = blk_off[t] - blk_off[b0]    # block offset in batch
                    for g in range(0, nb, 4):
                        gn = min(4, nb - g)
                        ps_q = psA.tile([P, 4, TD], F32, tag="psq")
                        for b in range(g, g + gn):
                            nc.tensor.matmul(
                                out=ps_q[:, b - g, :],
                                lhsT=ST_sb[:, (go + b) * P:(go + b + 1) * P],
                                rhs=qh_sb[:, t * TD:(t + 1) * TD],
                                start=True, stop=True)
                        with nc.allow_low_precision(reason="f16 qh rows"):
                            nc.scalar.activation(
                                out=qsb[:, go + g:go + g + gn, :].rearrange("p b f -> p (b f)"),
                                in_=ps_q[:, 0:gn, :].rearrange("p b f -> p (b f)"),
                                func=AF.Copy)

                # batched logits: prod = qh*kh, qk = sum_d, attn = qk + eb
                # (explicit wait: gather-DMA completion is NOT implied by the
                # prep's engine tick, so the first khv_g consumer must wait on
                # the descriptor-baked DMA semaphore)
                nc.vector.wait_ge(dma_sem, wait_thresh)
                prod = wpool.tile([P, MAXB, TD], F16, tag="prod")
                nc.vector.tensor_tensor(
                    out=prod[:, :nbb, :], in0=qsb[:, :nbb, :],
                    in1=khv_g[:, :nbb, 0:TD], op=ALU.mult)
                qk = wpool.tile([P, MAXB, H], F16, tag="qk")
                with nc.allow_low_precision(reason="f16 qk logits"):
                    nc.vector.reduce_sum(
                        out=qk[:, :nbb, :],
                        in_=prod[:, :nbb, :].rearrange("p b (h d) -> p b h d", h=H),
                        axis=mybir.AxisListType.X)
                attn = wpool.tile([P, MAXB * H], F16, tag="attn")
                with nc.allow_low_precision(reason="f16 logits"):
                    nc.vector.tensor_tensor(
                        out=attn[:, :nbb * H],
                        in0=qk[:, :nbb, :].rearrange("p b h -> p (b h)"),
                        in1=eb_sb[:, :nbb * H], op=ALU.add)
                w_t = wpool.tile([P, MAXB, H], F16, tag="w")
                with nc.allow_low_precision(reason="f16 weights"):
                    nc.scalar.activation(out=w_t[:, :nbb, :].rearrange("p b h -> p (b h)"),
                                         in_=attn[:, :nbb * H], func=AF.Exp)
                # rhs = [w*vh | w]
                wv = wpool.tile([P, MAXB, 136], F16, tag="wv")
                with nc.allow_low_precision(reason="f16 weighted values"):
                    nc.vector.tensor_tensor(
                        out=wv[:, :nbb, 0:TD].rearrange("p b (h d) -> p b h d", h=H),
                        in0=khv_g[:, :nbb, TD:2 * TD].rearrange("p b (h d) -> p b h d", h=H),
                        in1=w_t[:, :nbb, :, None].to_broadcast([P, nbb, H, D]),
                        op=ALU.mult)
                    nc.vector.tensor_copy(out=wv[:, :nbb, TD:TD + H], in_=w_t[:, :nbb, :])

                # scatter-accumulate into num|den psum per tile
                for t in range(b0, b1):
                    nb = bpt[t]
                    go = blk_off[t] - blk_off[b0]
                    ps_nd = psB.tile([P, 136], F32, tag="nd")
                    for b in range(nb):
                        nc.tensor.matmul(out=ps_nd[:],
                                         lhsT=S_sb[:, (go + b) * P:(go + b + 1) * P],
                                         rhs=wv[:, go + b, :],
                                         start=(b == 0), stop=(b == nb - 1))
                    nd_sb = opool.tile([P, 136], F16, tag="nd_sb")
                    with nc.allow_low_precision(reason="f16 num/den partials"):
                        nc.scalar.activation(out=nd_sb[:], in_=ps_nd[:], func=AF.Copy)
                    if t < H1:
                        nc.sync.dma_start(out=t_nd_a[t * P:(t + 1) * P, :], in_=nd_sb[:])
                    else:
                        nc.sync.dma_start(
                            out=t_nd_b[(t - H1) * P:(t - H1 + 1) * P, :], in_=nd_sb[:])

            # ---- tail: reduce-scatter + finalize (post-loop so the cc ops
            # never block the gpsimd descriptor-generation stream) ----
            reduce_scatter(t_nd_a, t_ndr_a)
            reduce_scatter(t_nd_b, t_ndr_b)
            for tf in range(HA):
                finalize(tf, t_ndr_a, tf)
            for t in range(HB):
                finalize(t, t_ndr_b, HA + t)

    nc.compile()
    return nc


_CACHE = {}
LAST_RUN = {}


def kernel(**inputs) -> np.ndarray:
    q = np.asarray(inputs["q"], np.float32)
    k = np.asarray(inputs["k"], np.float32)
    v = np.asarray(inputs["v"], np.float32)
    edges = np.asarray(inputs["edges"], np.float32)
    edge_index = np.asarray(inputs["edge_index"])
    Wq, Wk, Wv = inputs["Wq"], inputs["Wk"], inputs["Wv"]
    Wb, bb, Wo, bo = inputs["Wb"], inputs["bb"], inputs["Wo"], inputs["bo"]

    cores, consts, meta = prepare(q, k, v, edges, edge_index, Wq, Wk, Wv, Wb, bb, Wo, bo)
    N = meta["N"]
    ntr = meta["ntiles_row"]
    H1 = ((ntr // 2 + 1) // 2) * 2
    HA, HB = H1 // 2, (ntr - H1) // 2

    key = (q.shape, edges.shape, meta["NBLK"])
    if key not in _CACHE:
        _CACHE[key] = build_program(meta)
    nc = _CACHE[key]

    in_maps = []
    for core in range(R * C):
        m = dict(cores[core])
        m.update({kk: np.ascontiguousarray(vv) for kk, vv in consts.items()})
        in_maps.append({kk: np.ascontiguousarray(vv) for kk, vv in m.items()})

    import os
    if os.environ.get("KERNEL_SIM"):
        from concourse.bass_interp import MultiCoreSim
        sim = MultiCoreSim(nc, num_cores=R * C)
        for ci, core_sim in sim.cores.items():
            for name, arr in in_maps[ci].items():
                core_sim.tensor(name)[:] = arr
        sim.simulate(check_with_hw=False)
        results = [{"o_out": np.array(sim.cores[ci].tensor("o_out"))}
                   for ci in range(R * C)]
    else:
        trace = bool(os.environ.get("KERNEL_TRACE"))
        res = bass_utils.run_bass_kernel_spmd(nc, in_maps, core_ids=list(range(R * C)),
                                              trace=trace)
        LAST_RUN["res"] = res
        results = res.results

    # assemble: core (i, j) has tiles [j*HA, j*HA+HA) + [H1+j*HB, H1+j*HB+HB)
    out = np.zeros((meta["NPAD"], QD), np.float32)
    node_tile, node_slot = meta["node_tile"], meta["node_slot"]
    for i in range(R):
        for j in range(C):
            o = results[i * C + j]["o_out"].astype(np.float32)  # [(HA+HB)*P, QD]
            ta = j * HA
            out[(i * ntr + ta) * P:(i * ntr + ta + HA) * P] = o[0:HA * P]
            tb = H1 + j * HB
            out[(i * ntr + tb) * P:(i * ntr + tb + HB) * P] = o[HA * P:(HA + HB) * P]
    # map back to node ids
    full = np.zeros((N, QD), np.float32)
    rowpos = node_tile * P + node_slot
    full[:, :] = out[rowpos[np.arange(N)]]
    full += np.asarray(bo, np.float32)[None, :]
    # zero-degree nodes: reference yields bo
    zd = meta["deg"] == 0
    if zd.any():
        full[zd] = np.asarray(bo, np.float32)[None, :]
    return full


# revision 45
# speedup vs baseline: 1.1043x; 1.0024x over previous
"""GNN edge-softmax attention kernel for 8 Trainium2 NeuronCores.

Strategy (4 src-rows x 2 dst-halves core grid):
  - Host routes each edge to core (row(src), half(dst)). Nodes are packed
    into 128-node tiles balanced by edge count; each tile's edges are padded
    to whole 128-edge blocks so every core runs an identical program.
  - Host precomputes the edge-bias term eb = edges @ Wb.T + bb (tiny linear
    map) so the device never touches the raw 64-dim edge features.
  - Per core: project q/k/v slices with PE into f16 tables (khv table
    assembled across the quad with a 2-chunk AllGather overlapped with the
    qh projection), then for each gather batch: pipelined dma_gather
    (prepare_only + trigger_dma, so descriptor prep never blocks on DMA),
    per-tile one-hot matmuls for qh rows, batched DVE mult/reduce for the
    per-edge logits, exp on ACT, and scatter-accumulate num/den into PSUM
    with selection-matrix matmuls.
  - num/den partials are written in f16 and pair-ReduceScattered in two
    chunks (first chunk overlaps the main loop); each core normalizes and
    applies the output projection for its half of the tiles. Host adds bo.
"""

import math
import sys

import numpy as np

sys.path.insert(0, "/opt/trn_rl_repo")

import concourse.bacc as bacc
import concourse.bass as bass
import concourse.mybir as mybir
import concourse.tile as tile
from concourse import bass_utils

F16 = mybir.dt.float16
F8 = mybir.dt.float8e4
F32 = mybir.dt.float32
I16 = mybir.dt.int16

H = 8            # heads
D = 16           # head dim
TD = H * D       # 128
QD = 256         # q/k/v feature dim
PD = 64          # edge pair feature dim
R = 4            # src rows of the core grid
C = 2            # dst cols of the core grid
P = 128

AF = mybir.ActivationFunctionType
ALU = mybir.AluOpType


def _wrap16(idx: np.ndarray) -> np.ndarray:
    """dma_gather index layout: [128, n/16] with idx i at (i%16 + 16k, i//16)."""
    n = idx.shape[0]
    assert n % 16 == 0
    w = idx.reshape(n // 16, 16).T.astype(np.int16)  # [16, n/16]
    return np.tile(w, (8, 1))  # replicate across the 8 partition groups


def prepare(q, k, v, edges, edge_index, Wq, Wk, Wv, Wb, bb, Wo, bo):
    N = q.shape[0]
    E = edges.shape[0]
    ntiles_row = math.ceil(N / (R * P))          # tiles per src row
    NROW = ntiles_row * P                        # nodes per row (padded)
    NPAD = NROW * R
    DHALF = NPAD // 2                            # dst-half size
    assert DHALF < 32768, "dst half must fit int16"
    Q4 = DHALF // 4                              # rows projected per core
    NQ4 = Q4 // P
    CH0 = ((NQ4 + 1) // 2) * P                   # chunk-0 rows (per member)
    CH1 = Q4 - CH0                               # chunk-1 rows

    src = np.asarray(edge_index[:, 0], dtype=np.int64)
    dst = np.asarray(edge_index[:, 1], dtype=np.int64)
    deg = np.bincount(src, minlength=N)

    # --- greedy node->tile packing balanced by edge count ---
    T = R * ntiles_row
    order = np.argsort(-deg, kind="stable")
    tile_cnt = np.zeros(T, dtype=np.int64)       # nodes in tile
    tile_edges = np.zeros(T, dtype=np.int64)
    node_tile = np.zeros(N, dtype=np.int32)
    node_slot = np.zeros(N, dtype=np.int32)
    import heapq
    heap = [(0, t) for t in range(T)]
    heapq.heapify(heap)
    for n in order:
        while True:
            e_cnt, t = heapq.heappop(heap)
            if tile_cnt[t] < P:
                break
        node_tile[n] = t
        node_slot[n] = tile_cnt[t]
        tile_cnt[t] += 1
        tile_edges[t] += deg[n]
        if tile_cnt[t] < P:
            heapq.heappush(heap, (tile_edges[t], t))

    row_of_edge = node_tile[src] // ntiles_row
    j_of_edge = (dst // DHALF).astype(np.int64)
    tloc_of_edge = (node_tile[src] % ntiles_row).astype(np.int64)

    # per (core, tile_local) edge counts -> shared block counts per tile slot
    core_of_edge = row_of_edge * C + j_of_edge
    cnt = np.zeros((R * C, ntiles_row), dtype=np.int64)
    np.add.at(cnt, (core_of_edge, tloc_of_edge), 1)
    bpt = np.maximum(1, np.ceil(cnt.max(axis=0) / P).astype(np.int64))  # [ntiles_row]
    blk_off = np.concatenate([[0], np.cumsum(bpt)])   # block offset per tile
    NBLK = int(blk_off[-1])
    ECAP = NBLK * P

    # host-side edge bias: eb[e, h] = edges @ Wb.T + bb
    eb_all = (np.asarray(edges, np.float32) @ np.asarray(Wb, np.float32).T
              + np.asarray(bb, np.float32)[None, :]).astype(np.float16)  # [E, H]

    # khv table row for a dst node local to half j (chunk-major layout so the
    # AllGather can run in two chunks writing contiguous table regions)
    def table_row(loc):
        m = loc // Q4
        r = loc % Q4
        return np.where(r < CH0, m * CH0 + r, 4 * CH0 + m * CH1 + (r - CH0))

    # --- per-core edge arrays ---
    cores = []
    for core in range(R * C):
        i, j = core // C, core % C
        mask = core_of_edge == core
        es, ed, et = src[mask], dst[mask], tloc_of_edge[mask]
        # order edges by tile slot
        ordr = np.argsort(et, kind="stable")
        es, ed, et = es[ordr], ed[ordr], et[ordr]
        # positions: per tile, fill from blk_off[t]*P
        pos = np.zeros(len(es), dtype=np.int64)
        start = 0
        for t in range(ntiles_row):
            c = int((et == t).sum())
            pos[start:start + c] = blk_off[t] * P + np.arange(c)
            start += c
        eidx = np.nonzero(mask)[0][ordr]

        import ml_dtypes
        F8NP = ml_dtypes.float8_e4m3
        dst_local = np.zeros(ECAP, dtype=np.int16)
        src_rel = np.full(ECAP, 255, dtype=np.int64)
        ebE = np.zeros((ECAP, H), dtype=np.float16)
        dst_local[pos] = table_row(ed - j * DHALF).astype(np.int16)
        src_rel[pos] = node_slot[es]
        ebE[pos] = eb_all[eidx]
        # eb in edge-major block layout [128, NBLK*H]
        ebT = np.ascontiguousarray(
            ebE.reshape(NBLK, P, H).transpose(1, 0, 2)).reshape(P, NBLK * H)
        # one-hot selection matrices (fp8, exact 0/1)
        S_en = np.zeros((ECAP, P), dtype=F8NP)
        valid = src_rel < P
        S_en[np.nonzero(valid)[0], src_rel[valid]] = 1.0
        S_en3 = S_en.reshape(NBLK, P, P)                       # [b, e, n]
        S_mat = np.ascontiguousarray(S_en3.transpose(1, 0, 2)).reshape(P, ECAP)   # [e_part, (b n)]
        ST_mat = np.ascontiguousarray(S_en3.transpose(2, 0, 1)).reshape(P, ECAP)  # [n_part, (b e)]

        # constants: this core projects quarter i of half j's khv table
        qlo = j * DHALF + i * Q4
        qhi = min(qlo + Q4, N)
        kT = np.zeros((QD, Q4), dtype=np.float16)
        vT = np.zeros((QD, Q4), dtype=np.float16)
        if qhi > qlo:
            kT[:, :qhi - qlo] = np.asarray(k[qlo:qhi], np.float32).T.astype(np.float16)
            vT[:, :qhi - qlo] = np.asarray(v[qlo:qhi], np.float32).T.astype(np.float16)
        # q rows permuted into (tile_local, slot) order for this row i
        qT = np.zeros((QD, NROW), dtype=np.float16)
        rmask = node_tile // ntiles_row == i
        rn = np.nonzero(rmask)[0]
        qpos = (node_tile[rn] % ntiles_row) * P + node_slot[rn]
        qT[:, qpos] = np.asarray(q[rn], np.float32).T.astype(np.float16)

        cores.append(dict(
            dst_idx=_wrap16(dst_local), S_mat=S_mat, ST_mat=ST_mat,
            ebT=ebT, kT=kT, vT=vT, qT=qT,
        ))

    norm = D ** -0.5
    consts = dict(
        WkT=np.asarray(Wk, np.float32).T.astype(np.float16),
        WvT=np.asarray(Wv, np.float32).T.astype(np.float16),
        WqT=(np.asarray(Wq, np.float32) * norm).T.astype(np.float16),
        WoT=np.asarray(Wo, np.float32).T.astype(np.float16),
        identity=np.eye(P, dtype=np.float16),
    )
    meta = dict(N=N, NPAD=NPAD, NROW=NROW, DHALF=DHALF, ntiles_row=ntiles_row,
                NBLK=NBLK, ECAP=ECAP, bpt=bpt.tolist(), blk_off=blk_off.tolist(),
                CH0=CH0, CH1=CH1,
                node_tile=node_tile, node_slot=node_slot, deg=deg)
    return cores, consts, meta


def build_program(meta, gather_batch=3):
    """Build the SPMD bass program. Returns compiled nc."""
    ntr = meta["ntiles_row"]
    NROW, DHALF = meta["NROW"], meta["DHALF"]
    NBLK, ECAP = meta["NBLK"], meta["ECAP"]
    bpt, blk_off = meta["bpt"], meta["blk_off"]
    CH0, CH1 = meta["CH0"], meta["CH1"]
    Q4 = DHALF // 4
    NQ = NROW // P        # qh chunks
    # ReduceScatter split: chunk A covers tiles [0, H1), chunk B the rest.
    H1 = ((ntr // 2 + 1) // 2) * 2  # even tile count near the middle
    HA, HB = H1 // 2, (ntr - H1) // 2
    assert H1 % 2 == 0 and (ntr - H1) % 2 == 0

    GB = gather_batch
    # gather batches group consecutive tiles
    batches = []
    t0 = 0
    while t0 < ntr:
        t1 = min(t0 + GB, ntr)
        batches.append((t0, t1))
        t0 = t1
    MAXB = max(blk_off[b1] - blk_off[b0] for b0, b1 in batches)

    nc = bacc.Bacc("TRN2", target_bir_lowering=False, debug=False, num_devices=R * C)
    dt = nc.dram_tensor
    # inputs
    t_dst = dt("dst_idx", [P, ECAP // 16], I16, kind="ExternalInput").ap()
    t_S = dt("S_mat", [P, ECAP], F8, kind="ExternalInput").ap()
    t_ST = dt("ST_mat", [P, ECAP], F8, kind="ExternalInput").ap()
    t_eb = dt("ebT", [P, NBLK * H], F16, kind="ExternalInput").ap()
    t_kT = dt("kT", [QD, Q4], F16, kind="ExternalInput").ap()
    t_vT = dt("vT", [QD, Q4], F16, kind="ExternalInput").ap()
    t_qT = dt("qT", [QD, NROW], F16, kind="ExternalInput").ap()
    t_WkT = dt("WkT", [QD, TD], F16, kind="ExternalInput").ap()
    t_WvT = dt("WvT", [QD, TD], F16, kind="ExternalInput").ap()
    t_WqT = dt("WqT", [QD, TD], F16, kind="ExternalInput").ap()
    t_WoT = dt("WoT", [TD, QD], F16, kind="ExternalInput").ap()
    t_id = dt("identity", [P, P], F16, kind="ExternalInput").ap()
    # internal DRAM (chunked tensors are separate so Tile's tensor-granular
    # dependency tracking doesn't serialize later writes behind collectives)
    t_khv = dt("khv_tab", [DHALF, 2 * TD], F16).ap()
    t_khv_h0 = dt("khv_half0", [CH0, 2 * TD], F16).ap()
    t_khv_h1 = dt("khv_half1", [CH1, 2 * TD], F16).ap()
    t_nd_a = dt("nd_part_a", [H1 * P, 136], F16).ap()
    t_nd_b = dt("nd_part_b", [(ntr - H1) * P, 136], F16).ap()
    t_ndr_a = dt("nd_red_a", [HA * P, 136], F16).ap()
    t_ndr_b = dt("nd_red_b", [HB * P, 136], F16).ap()
    # output: core (i, j) finalizes tiles [j*HA, j*HA+HA) and
    # [H1 + j*HB, H1 + j*HB + HB) of its row
    t_out = dt("o_out", [(HA + HB) * P, QD], F16, kind="ExternalOutput").ap()

    dma_sem = nc.alloc_semaphore("swdge_dma")

    with tile.TileContext(nc) as tc:
        with (
            tc.tile_pool(name="const", bufs=1) as cpool,
            tc.tile_pool(name="proj", bufs=3) as ppool,
            tc.tile_pool(name="gath", bufs=3) as gpool,
            tc.tile_pool(name="work", bufs=2) as wpool,
            tc.tile_pool(name="out", bufs=2) as opool,
            tc.tile_pool(name="psA", bufs=2, space="PSUM") as psA,
            tc.tile_pool(name="psB", bufs=2, space="PSUM") as psB,
            tc.tile_pool(name="psC", bufs=2, space="PSUM") as psC,
        ):
            # ---- constants to SBUF ----
            c_WkT = cpool.tile([P, 2 * TD], F16)
            nc.sync.dma_start(out=c_WkT[:, 0:TD], in_=t_WkT[0:P, :])
            nc.sync.dma_start(out=c_WkT[:, TD:2 * TD], in_=t_WkT[P:QD, :])
            c_WvT = cpool.tile([P, 2 * TD], F16)
            nc.sync.dma_start(out=c_WvT[:, 0:TD], in_=t_WvT[0:P, :])
            nc.sync.dma_start(out=c_WvT[:, TD:2 * TD], in_=t_WvT[P:QD, :])
            c_WqT = cpool.tile([P, 2 * TD], F16)
            nc.sync.dma_start(out=c_WqT[:, 0:TD], in_=t_WqT[0:P, :])
            nc.sync.dma_start(out=c_WqT[:, TD:2 * TD], in_=t_WqT[P:QD, :])
            c_WoT = cpool.tile([TD, QD], F16); nc.sync.dma_start(out=c_WoT[:], in_=t_WoT)
            c_id = cpool.tile([P, P], F16); nc.sync.dma_start(out=c_id[:], in_=t_id)
            c_dsti = cpool.tile([P, ECAP // 16], I16)
            nc.sync.dma_start(out=c_dsti[:], in_=t_dst)
            qh_sb = cpool.tile([P, NQ * TD], F16)

            # ---- phase A: projections (qh -> SBUF table, khv -> HBM) ----
            def project_qh():
                for g0 in range(0, NQ, 8):
                    g1 = min(g0 + 8, NQ)
                    w = (g1 - g0) * P
                    ina = ppool.tile([P, 1024], F16, tag="ina")
                    inb = ppool.tile([P, 1024], F16, tag="inb")
                    nc.sync.dma_start(out=ina[:, :w], in_=t_qT[0:P, g0 * P:g0 * P + w])
                    nc.sync.dma_start(out=inb[:, :w], in_=t_qT[P:QD, g0 * P:g0 * P + w])
                    for cc in range(g0, g1):
                        o = (cc - g0) * P
                        ps = psA.tile([P, 4, TD], F32, tag="psq")
                        nc.tensor.matmul(out=ps[:, 0, :], lhsT=ina[:, o:o + P],
                                         rhs=c_WqT[:, 0:TD], start=True, stop=False)
                        nc.tensor.matmul(out=ps[:, 0, :], lhsT=inb[:, o:o + P],
                                         rhs=c_WqT[:, TD:2 * TD], start=False, stop=True)
                        nc.scalar.activation(out=qh_sb[:, cc * TD:(cc + 1) * TD],
                                             in_=ps[:, 0, :], func=AF.Copy)

            # khv table: interleave kh|vh per node row; each core projects its
            # quarter of the half, 2-chunk AllGather assembles t_khv.
            def project_khv(r0, r1, t_half):
                for g0 in range(r0 // P, r1 // P, 8):
                    g1 = min(g0 + 8, r1 // P)
                    w = (g1 - g0) * P
                    ka = ppool.tile([P, 1024], F16, tag="ka")
                    kb = ppool.tile([P, 1024], F16, tag="kb")
                    va = ppool.tile([P, 1024], F16, tag="va")
                    vb = ppool.tile([P, 1024], F16, tag="vb")
                    nc.sync.dma_start(out=ka[:, :w], in_=t_kT[0:P, g0 * P:g0 * P + w])
                    nc.sync.dma_start(out=kb[:, :w], in_=t_kT[P:QD, g0 * P:g0 * P + w])
                    nc.sync.dma_start(out=va[:, :w], in_=t_vT[0:P, g0 * P:g0 * P + w])
                    nc.sync.dma_start(out=vb[:, :w], in_=t_vT[P:QD, g0 * P:g0 * P + w])
                    stage = ppool.tile([P, 8 * 2 * TD], F16, tag="kvstage")
                    for cc in range(g0, g1):
                        o = (cc - g0) * P
                        ps = psA.tile([P, 4, TD], F32, tag="psq")
                        nc.tensor.matmul(out=ps[:, 0, :], lhsT=ka[:, o:o + P],
                                         rhs=c_WkT[:, 0:TD], start=True, stop=False)
                        nc.tensor.matmul(out=ps[:, 0, :], lhsT=kb[:, o:o + P],
                                         rhs=c_WkT[:, TD:2 * TD], start=False, stop=True)
                        nc.tensor.matmul(out=ps[:, 1, :], lhsT=va[:, o:o + P],
                                         rhs=c_WvT[:, 0:TD], start=True, stop=False)
                        nc.tensor.matmul(out=ps[:, 1, :], lhsT=vb[:, o:o + P],
                                         rhs=c_WvT[:, TD:2 * TD], start=False, stop=True)
                        nc.scalar.activation(
                            out=stage[:, (cc - g0) * 256:(cc - g0) * 256 + 256],
                            in_=ps[:, 0:2, :].rearrange("p b f -> p (b f)"),
                            func=AF.Copy)
                    nc.sync.dma_start(
                        out=t_half[(g0 - r0 // P) * P:(g1 - r0 // P) * P, :].rearrange(
                            "(c p) w -> p c w", p=P),
                        in_=stage[:, :(g1 - g0) * 256].rearrange(
                            "p (c w) -> p c w", w=256))

            quads = [[0, 2, 4, 6], [1, 3, 5, 7]]
            project_khv(0, CH0, t_khv_h0)
            nc.gpsimd.collective_compute(
                "AllGather", ALU.bypass, replica_groups=quads,
                ins=[t_khv_h0], outs=[t_khv[0:4 * CH0, :]])
            project_khv(CH0, Q4, t_khv_h1)
            nc.gpsimd.collective_compute(
                "AllGather", ALU.bypass, replica_groups=quads,
                ins=[t_khv_h1], outs=[t_khv[4 * CH0:DHALF, :]])
            project_qh()

            pairs = [[0, 1], [2, 3], [4, 5], [6, 7]]

            def reduce_scatter(t_in, t_out_):
                with nc.allow_low_precision(reason="f16 num/den partials"):
                    nc.gpsimd.collective_compute(
                        "ReduceScatter", ALU.add, replica_groups=pairs,
                        ins=[t_in], outs=[t_out_])

            # ---- finalize: normalize + output projection for my tiles ----
            def finalize(t, src_nd, out_tile):
                ndl = opool.tile([P, 136], F16, tag="ndl")
                nc.sync.dma_start(out=ndl[:], in_=src_nd[t * P:(t + 1) * P, :])
                rden = opool.tile([P, H], F32, tag="rden")
                # +eps in f32 so empty node slots yield 0 instead of 0*inf=NaN
                # (a NaN row would poison the whole tile through the PE transpose)
                nc.vector.tensor_scalar_add(out=rden[:], in0=ndl[:, TD:TD + H],
                                            scalar1=1e-30)
                nc.vector.reciprocal(out=rden[:], in_=rden[:])
                o_sb = opool.tile([P, TD], F16, tag="o_sb")
                with nc.allow_low_precision(reason="f16 normalized output"):
                    nc.vector.tensor_tensor(
                        out=o_sb[:].rearrange("p (h d) -> p h d", h=H),
                        in0=ndl[:, 0:TD].rearrange("p (h d) -> p h d", h=H),
                        in1=rden[:, :, None].to_broadcast([P, H, D]),
                        op=ALU.mult)
                ps_oT = psC.tile([P, P], F16, tag="oT")
                nc.tensor.transpose(out=ps_oT[:], in_=o_sb[:], identity=c_id[:])
                oT_sb = opool.tile([P, P], F16, tag="oT_sb")
                nc.scalar.activation(out=oT_sb[:], in_=ps_oT[:], func=AF.Copy)
                ps_o = psC.tile([P, QD], F32, tag="ps_o")
                nc.tensor.matmul(out=ps_o[:], lhsT=oT_sb[:], rhs=c_WoT[:],
                                 start=True, stop=True)
                out_sb = opool.tile([P, QD], F16, tag="out_sb")
                with nc.allow_low_precision(reason="f16 output"):
                    nc.scalar.activation(out=out_sb[:], in_=ps_o[:], func=AF.Copy)
                nc.sync.dma_start(out=t_out[out_tile * P:(out_tile + 1) * P, :],
                                  in_=out_sb[:])

            # ---- phase M: main loop over gather batches ----
            n_subg = [0]

            for bidx, (b0, b1) in enumerate(batches):
                e0, e1 = blk_off[b0] * P, blk_off[b1] * P
                ne = e1 - e0
                nbb = ne // P
                khv_g = gpool.tile([P, MAXB, 2 * TD], F16, tag="khv_g")
                # prepare_only + trigger_dma: descriptor generation (the Q7
                # bottleneck, ~8ns/edge) pipelines with the transfers
                nc.gpsimd.dma_gather(
                    out_ap=khv_g[:, :nbb, :], in_ap=t_khv,
                    idxs_ap=c_dsti[:, e0 // 16:e1 // 16],
                    num_idxs=ne, num_idxs_reg=ne, elem_size=2 * TD,
                    single_packet=False, prepare_only=True, sem=dma_sem)
                nc.gpsimd.trigger_dma(count=None)
                n_subg[0] += 1
                wait_thresh = 16 * n_subg[0]
                S_sb = gpool.tile([P, MAXB * P], F8, tag="S_sb")
                nc.sync.dma_start(out=S_sb[:, :ne], in_=t_S[:, e0:e1])
                ST_sb = gpool.tile([P, MAXB * P], F8, tag="ST_sb")
                nc.sync.dma_start(out=ST_sb[:, :ne], in_=t_ST[:, e0:e1])
                eb_sb = gpool.tile([P, MAXB * H], F16, tag="eb_sb")
                nc.sync.dma_start(out=eb_sb[:, :nbb * H],
                                  in_=t_eb[:, blk_off[b0] * H:blk_off[b1] * H])

                # qh rows via one-hot matmuls, copied to f16 working buffer
                qsb = wpool.tile([P, MAXB, TD], F16, tag="qsb")
                for t in range(b0, b1):
                    nb = bpt[t]
                    go = blk_off[t] - blk_off[b0]    # block offset in batch
                    for g in range(0, nb, 4):
                        gn = min(4, nb - g)
                        ps_q = psA.tile([P, 4, TD], F32, tag="psq")
                        for b in range(g, g + gn):
                            nc.tensor.matmul(
                                out=ps_q[:, b - g, :],
                                lhsT=ST_sb[:, (go + b) * P:(go + b + 1) * P],
                                rhs=qh_sb[:, t * TD:(t + 1) * TD],
                                start=True, stop=True)
                        with nc.allow_low_precision(reason="f16 qh rows"):
                            nc.scalar.activation(
                                out=qsb[:, go + g:go + g + gn, :].rearrange("p b f -> p (b f)"),
                                in_=ps_q[:, 0:gn, :].rearrange("p b f -> p (b f)"),
                                func=AF.Copy)

                # batched logits: prod = qh*kh, qk = sum_d, attn = qk + eb
                # (explicit wait: gather-DMA completion is NOT implied by the
                # prep's engine tick, so the first khv_g consumer must wait on
                # the descriptor-baked DMA semaphore)
                nc.vector.wait_ge(dma_sem, wait_thresh)
                prod = wpool.tile([P, MAXB, TD], F16, tag="prod")
                nc.vector.tensor_tensor(
                    out=prod[:, :nbb, :], in0=qsb[:, :nbb, :],
                    in1=khv_g[:, :nbb, 0:TD], op=ALU.mult)
                qk = wpool.tile([P, MAXB, H], F16, tag="qk")
                with nc.allow_low_precision(reason="f16 qk logits"):
                    nc.vector.reduce_sum(
                        out=qk[:, :nbb, :],
                        in_=prod[:, :nbb, :].rearrange("p b (h d) -> p b h d", h=H),
                        axis=mybir.AxisListType.X)
                attn = wpool.tile([P, MAXB * H], F16, tag="attn")
                with nc.allow_low_precision(reason="f16 logits"):
                    nc.vector.tensor_tensor(
                        out=attn[:, :nbb * H],
                        in0=qk[:, :nbb, :].rearrange("p b h -> p (b h)"),
                        in1=eb_sb[:, :nbb * H], op=ALU.add)
                w_t = wpool.tile([P, MAXB, H], F16, tag="w")
                with nc.allow_low_precision(reason="f16 weights"):
                    nc.scalar.activation(out=w_t[:, :nbb, :].rearrange("p b h -> p (b h)"),
                                         in_=attn[:, :nbb * H], func=AF.Exp)
                # rhs = [w*vh | w]
                wv = wpool.tile([P, MAXB, 136], F16, tag="wv")
                with nc.allow_low_precision(reason="f16 weighted values"):
                    nc.vector.tensor_tensor(
                        out=wv[:, :nbb, 0:TD].rearrange("p b (h d) -> p b h d", h=H),
                        in0=khv_g[:, :nbb, TD:2 * TD].rearrange("p b (h d) -> p b h d", h=H),
                        in1=w_t[:, :nbb, :, None].to_broadcast([P, nbb, H, D]),
                        op=ALU.mult)
                    nc.vector.tensor_copy(out=wv[:, :nbb, TD:TD + H], in_=w_t[:, :nbb, :])

                # scatter-accumulate into num|den psum per tile
                for t in range(b0, b1):
                    nb = bpt[t]
                    go = blk_off[t] - blk_off[b0]
                    ps_nd = psB.tile([P, 136], F32, tag="nd")
                    for b in range(nb):
                        nc.tensor.matmul(out=ps_nd[:],
                                         lhsT=S_sb[:, (go + b) * P:(go + b + 1) * P],
                                         rhs=wv[:, go + b, :],
                                         start=(b == 0), stop=(b == nb - 1))
                    nd_sb = opool.tile([P, 136], F16, tag="nd_sb")
                    with nc.allow_low_precision(reason="f16 num/den partials"):
                        nc.scalar.activation(out=nd_sb[:], in_=ps_nd[:], func=AF.Copy)
                    if t < H1:
                        nc.sync.dma_start(out=t_nd_a[t * P:(t + 1) * P, :], in_=nd_sb[:])
                    else:
                        nc.sync.dma_start(
                            out=t_nd_b[(t - H1) * P:(t - H1 + 1) * P, :], in_=nd_sb[:])

            # ---- tail: reduce-scatter + finalize (post-loop so the cc ops
            # never block the gpsimd descriptor-generation stream) ----
            reduce_scatter(t_nd_a, t_ndr_a)
            reduce_scatter(t_nd_b, t_ndr_b)
            for tf in range(HA):
                finalize(tf, t_ndr_a, tf)
            for t in range(HB):
                finalize(t, t_ndr_b, HA + t)

    nc.compile()
    return nc


_CACHE = {}
LAST_RUN = {}


def kernel(**inputs) -> np.ndarray:
    q = np.asarray(inputs["q"], np.float32)
    k = np.asarray(inputs["k"], np.float32)
    v = np.asarray(inputs["v"], np.float32)
    edges = np.asarray(inputs["edges"], np.float32)
    edge_index = np.asarray(inputs["edge_index"])
    Wq, Wk, Wv = inputs["Wq"], inputs["Wk"], inputs["Wv"]
    Wb, bb, Wo, bo = inputs["Wb"], inputs["bb"], inputs["Wo"], inputs["bo"]

    cores, consts, meta = prepare(q, k, v, edges, edge_index, Wq, Wk, Wv, Wb, bb, Wo, bo)
    N = meta["N"]
    ntr = meta["ntiles_row"]
    H1 = ((ntr // 2 + 1) // 2) * 2
    HA, HB = H1 // 2, (ntr - H1) // 2

    key = (q.shape, edges.shape, meta["NBLK"])
    if key not in _CACHE:
        _CACHE[key] = build_program(meta)
    nc = _CACHE[key]

    in_maps = []
    for core in range(R * C):
        m = dict(cores[core])
        m.update({kk: np.ascontiguousarray(vv) for kk, vv in consts.items()})
        in_maps.append({kk: np.ascontiguousarray(vv) for kk, vv in m.items()})

    import os
    if os.environ.get("KERNEL_SIM"):
        from concourse.bass_interp import MultiCoreSim
        sim = MultiCoreSim(nc, num_cores=R * C)
        for ci, core_sim in sim.cores.items():
            for name, arr in in_maps[ci].items():
                core_sim.tensor(name)[:] = arr
        sim.simulate(check_with_hw=False)
        results = [{"o_out": np.array(sim.cores[ci].tensor("o_out"))}
                   for ci in range(R * C)]
    else:
        trace = bool(os.environ.get("KERNEL_TRACE"))
        res = bass_utils.run_bass_kernel_spmd(nc, in_maps, core_ids=list(range(R * C)),
                                              trace=trace)
        LAST_RUN["res"] = res
        results = res.results

    # assemble: core (i, j) has tiles [j*HA, j*HA+HA) + [H1+j*HB, H1+j*HB+HB)
    out = np.zeros((meta["NPAD"], QD), np.float32)
    node_tile, node_slot = meta["node_tile"], meta["node_slot"]
    for i in range(R):
        for j in range(C):
            o = results[i * C + j]["o_out"].astype(np.float32)  # [(HA+HB)*P, QD]
            ta = j * HA
            out[(i * ntr + ta) * P:(i * ntr + ta + HA) * P] = o[0:HA * P]
            tb = H1 + j * HB
            out[(i * ntr + tb) * P:(i * ntr + tb + HB) * P] = o[HA * P:(HA + HB) * P]
    # map back to node ids
    full = np.zeros((N, QD), np.float32)
    rowpos = node_tile * P + node_slot
    full[:, :] = out[rowpos[np.arange(N)]]
    full += np.asarray(bo, np.float32)[None, :]
    # zero-degree nodes: reference yields bo
    zd = meta["deg"] == 0
    if zd.any():
        full[zd] = np.asarray(bo, np.float32)[None, :]
    return full
